# revision 1
# baseline (speedup 1.0000x reference)
"""Trainium2 Bass kernel for ANI per-species MLP (MoE routing).

Strategy (v2):
- MoE dispatch on host: gather atoms by species.  Species-pure cores:
  core c handles half of species c//2's atoms, so each core loads ONE
  species' weights (0.33MB vs 1.31MB replicated) -> less DMA, earlier
  first matmul.
- Grouped GEMMs per 512-atom chunk, hidden dim on SBUF partitions so
  per-partition bias/ReLU epilogues fuse the PSUM->SBUF copy; bf16 with
  fp32 PSUM accumulation (rel err vs fp32 reference ~2e-3).
- Software-pipelined emission: L1(ci+1) before L2(ci) so the PE never
  head-of-line blocks on an epilogue.
- 2 HWDGE queues (sync + scalar) stream x and carry the output blocks;
  outputs are h2 activations (192 valid rows, bf16) DMA'd per
  chunk-pair; layer 3 (the tiny W2 dot) + molecule scatter-add on host.
- Back-to-back dummy matmuls bridge body-start -> first-data: the HAM
  clock ramp follows SUSTAINED PE activity (measured: full clock ~4.4us
  after saturated onset vs ~10-12us sparse), so saturating the PE early
  buys full clock for the real work.
- Lean TileContext teardown (drain + sem-only barrier).
"""

import numpy as np
import ml_dtypes

from concourse import bacc
import concourse.mybir as mybir
from concourse import tile
from concourse.bass_utils import run_bass_kernel_spmd
from concourse.vector_clock import ScopedClock


class _LeanTileContext(tile.TileContext):
    """TileContext with a lean kernel tail (drain + one barrier only).

    The stock teardown's per-semaphore clears are redundant for a single
    top-level TileContext: the Bass preamble dma_reset/sem_clears the
    full kernel semaphore range at the start of every execution.
    """

    def _drain_and_barrier(self, tick_clock, wait_clock):
        drain_inst = self.nc.sync.drain()
        wait_clock.add_sem_waits(
            drain_inst.ins, ScopedClock({None: tick_clock.global_clock})
        )
        self.nc.all_engine_barrier(sem_only=True)
        popped = self.nc._tile_sem_poison_stack.pop()
        assert popped is self._sem_poison


B, A, D, E = 32, 1024, 384, 4
H1, H2 = 256, 192
NCORES = 8
DCH = D // 128          # 3 chunks of descriptor dim
H1CH = H1 // 128        # 2 chunks of hidden-1
H2CH = 2                # hidden-2 padded 192 -> 256 = 2 chunks
MAX_N = 512
W1_COLS = H1CH * DCH * 128      # 768, h-major: c = h*384 + d*128
WH_COLS = H2CH * H1CH * 128     # 512, m-major: c = m*256 + k*128
WT_COLS = W1_COLS + WH_COLS     # 1280
BF16 = ml_dtypes.bfloat16

_graph_cache = {}
_last_run = {}


def _chunks(cap):
    """Chunk plan: runt first, then a 256 ramp chunk (early DMA
    throughput is low, so a small second chunk lands sooner), uniform
    512s, and a 256 tail chunk (small final output block)."""
    sizes = []
    rem = cap
    if rem % MAX_N:
        sizes.append(rem % MAX_N)
        rem -= rem % MAX_N
    for tail in (256,):
        if rem >= MAX_N + 2 * tail:
            sizes.append(tail)
            rem -= 2 * tail
    ntail = len(sizes) - (1 if sizes and sizes[0] != 256 else 0)
    while rem >= MAX_N:
        sizes.append(MAX_N)
        rem -= MAX_N
    if rem > 0:
        sizes.append(rem)
    if ntail:
        sizes.append(256)
    out, n0 = [], 0
    for n in sizes:
        out.append((n0, n))
        n0 += n
    return out


def _out_blocks(chunk_list):
    """Output-DMA blocks: the last two chunks go out alone (short tail
    chain); earlier chunks pair up, first block absorbing the odd one."""
    k = len(chunk_list)
    bounds = sorted(set(list(range(k, 0, -2)) + ([k - 1] if k > 1 else [])))
    blocks, start_ci = [], 0
    for end_ci in bounds:
        if end_ci <= start_ci:
            continue
        s = chunk_list[start_ci][0]
        last = chunk_list[end_ci - 1]
        blocks.append((end_ci - 1, s, last[0] + last[1]))
        start_ci = end_ci
    return blocks


def _build_graph(cap):
    f32, bf = mybir.dt.float32, mybir.dt.bfloat16
    Act = mybir.ActivationFunctionType
    chunk_list = _chunks(cap)
    out_blocks = _out_blocks(chunk_list)
    out_after = {ci: (s, e) for ci, s, e in out_blocks}

    nc = bacc.Bacc()
    xt_d = nc.declare_dram_parameter("xt", [128, DCH * cap], bf, isOutput=False)
    wt_d = nc.declare_dram_parameter("wt", [128, WT_COLS], bf, isOutput=False)
    bias_d = nc.declare_dram_parameter("bias", [128, 4, 1], f32, isOutput=False)
    out0_d = nc.declare_dram_parameter("out0", [128, cap], bf, isOutput=True)
    out1_d = nc.declare_dram_parameter("out1", [64, cap], bf, isOutput=True)

    with _LeanTileContext(nc) as tc:
        with (
            tc.tile_pool(name="wp", bufs=1) as wp,
            tc.tile_pool(name="xp", bufs=1) as xp,
            tc.tile_pool(name="hp", bufs=1) as hp,
            tc.tile_pool(name="pp", bufs=1, space="PSUM") as pp,
        ):
            wt_s = wp.tile([128, WT_COLS], bf, tag="wt")
            bias_s = wp.tile([128, 4, 1], f32, tag="bias")
            x_ts = [
                xp.tile([128, DCH, n], bf, tag=f"x_{ci}", name=f"x_{ci}")
                for ci, (n0, n) in enumerate(chunk_list)
            ]
            h1b = [
                hp.tile([128, H1CH, MAX_N], bf, tag=f"h1_{b}", name=f"h1_{b}")
                for b in range(2)
            ]
            h2_all = hp.tile([128, H2CH, cap], bf, tag="h2")
            ph1 = [
                [pp.tile([128, MAX_N], f32, tag=f"ph1_{h}_{b}", name=f"ph1_{h}_{b}")
                 for b in range(2)]
                for h in range(H1CH)
            ]
            ph2 = [
                [pp.tile([128, MAX_N], f32, tag=f"ph2_{m}_{b}", name=f"ph2_{m}_{b}")
                 for b in range(2)]
                for m in range(H2CH)
            ]

            # PE warm-up bridge: back-to-back dummy matmuls (no DMA deps)
            # from body start until real data arrives (~10us).  The HAM
            # clock ramp follows SUSTAINED activity (measured: full clock
            # ~4.4us after saturated onset vs ~10-12us with sparse
            # activity), so saturating the PE here buys full clock for the
            # real work almost immediately.
            warm_t = wp.tile([128, MAX_N], bf, tag="warm")
            nc.vector.memset(warm_t[:], 0.0)
            for _ in range(8):
                nc.tensor.matmul(
                    ph2[1][1][0:1, 0:MAX_N], lhsT=warm_t[:, 0:1],
                    rhs=warm_t[:, 0:MAX_N], start=True, stop=True,
                )

            # --- input DMAs ---
            # sync: weight pieces + bias + odd x chunks; scalar: even x
            # chunks (interleaved into the loop so scalar's epilogues are
            # not queued behind a block of DMA issues).  The first two
            # (small ramp) chunks' epilogues run on vector so the scalar
            # ACT_TABLE_LOAD (~1.3us, auto-inserted before the first
            # ACTIVATE) is off the critical path.
            def x_dma(ci):
                if ci >= len(chunk_list):
                    return
                n0, n = chunk_list[ci]
                c0 = DCH * n0
                # x0 (tiny runt) rides sync behind only w1h0; the rest
                # alternate so bytes balance across the two queues.
                eng = nc.sync if (ci % 2 == 1 or ci == 0) else nc.scalar
                eng.dma_start(out=x_ts[ci][:], in_=xt_d[:, c0:c0 + DCH * n])

            # Both W1 halves land first, one per HWDGE queue in parallel
            # (layer 1 needs BOTH halves for every chunk); Wh follows on
            # scalar; bias on gpsimd frees queue slots.  Early DMA
            # throughput is ~115GB/s per queue, so ordering here sets
            # when the PE can leave the warm-up dummies.
            nc.sync.dma_start(out=wt_s[:, :W1_COLS // 2], in_=wt_d[:, :W1_COLS // 2])
            nc.scalar.dma_start(
                out=wt_s[:, W1_COLS // 2:W1_COLS], in_=wt_d[:, W1_COLS // 2:W1_COLS]
            )
            x_dma(0)
            nc.scalar.dma_start(out=wt_s[:, W1_COLS:], in_=wt_d[:, W1_COLS:])
            x_dma(1)
            nc.gpsimd.dma_start(out=bias_s[:], in_=bias_d[:])

            # --- compute, software-pipelined: L1(ci+1) is emitted before
            # L2(ci) so the PE never head-of-line blocks on chunk ci's
            # h1 epilogue ---
            def layer1(ci):
                n0, n = chunk_list[ci]
                x_t = x_ts[ci]
                b = ci % 2
                h1_t = h1b[b]
                for h in range(H1CH):
                    ps = ph1[h][b][:, :n]
                    for d in range(DCH):
                        c0 = h * (DCH * 128) + d * 128
                        nc.tensor.matmul(
                            ps,
                            lhsT=wt_s[:, c0:c0 + 128],
                            rhs=x_t[:, d, :],
                            start=(d == 0),
                            stop=(d == DCH - 1),
                        )
                    if h == 0:
                        nc.scalar.activation(
                            h1_t[:, h, :n], ps, Act.Identity,
                            bias=bias_s[:, 0, :],
                        )
                    else:
                        nc.vector.tensor_scalar_add(
                            h1_t[:, h, :n], ps, bias_s[:, h, :]
                        )

            # layer 2: h2.T = relu(Wh.T @ h1.T + bh), + output block DMA
            def layer2(ci):
                n0, n = chunk_list[ci]
                b = ci % 2
                h1_t = h1b[b]
                for m in range(H2CH):
                    ps = ph2[m][b][:, :n]
                    for k in range(H1CH):
                        c0 = W1_COLS + m * (H1CH * 128) + k * 128
                        nc.tensor.matmul(
                            ps,
                            lhsT=wt_s[:, c0:c0 + 128],
                            rhs=h1_t[:, k, :n],
                            start=(k == 0),
                            stop=(k == H1CH - 1),
                        )
                    if m == 0:
                        nc.scalar.activation(
                            h2_all[:, m, n0:n0 + n], ps, Act.Relu,
                            bias=bias_s[:, 2, :],
                        )
                    else:
                        nc.vector.tensor_scalar(
                            h2_all[:, m, n0:n0 + n], ps, bias_s[:, 2 + m, :],
                            0.0, mybir.AluOpType.add, mybir.AluOpType.max,
                        )
                if ci in out_after:
                    s, e = out_after[ci]
                    blk = list(out_after).index(ci)
                    eng0 = nc.sync if blk % 2 == 0 else nc.scalar
                    eng1 = nc.scalar if blk % 2 == 0 else nc.sync
                    eng0.dma_start(out=out0_d[:, s:e], in_=h2_all[:, 0, s:e])
                    eng1.dma_start(out=out1_d[:, s:e], in_=h2_all[0:64, 1, s:e])

            nchunks = len(chunk_list)
            for ci in range(nchunks):
                x_dma(ci + 2)
                layer1(ci)
                if ci >= 1:
                    layer2(ci - 1)
            layer2(nchunks - 1)
    return nc


def _pack_weights_one(W1e, Whe):
    """[128, WT_COLS] bf16 blob for one species: w1 h-major | wh m-major."""
    w1p = W1e.reshape(DCH, 128, H1CH, 128).transpose(1, 2, 0, 3).reshape(
        128, W1_COLS
    )
    whpad = np.zeros((H1, H2CH * 128), np.float32)
    whpad[:, :H2] = Whe
    whp = whpad.reshape(H1CH, 128, H2CH, 128).transpose(1, 2, 0, 3).reshape(
        128, WH_COLS
    )
    return np.concatenate([w1p, whp], axis=1).astype(BF16)


def kernel(representation, species, W1, b1, Wh, bh, W2, b2):
    rep = np.ascontiguousarray(np.asarray(representation, np.float32)).reshape(
        B * A, D
    )
    spec = np.asarray(species).reshape(B * A)
    W1 = np.asarray(W1, np.float32)
    b1 = np.asarray(b1, np.float32)
    Wh = np.asarray(Wh, np.float32)
    bh = np.asarray(bh, np.float32)
    W2 = np.asarray(W2, np.float32)
    b2 = np.asarray(b2, np.float32)

    # dispatch: species-pure cores; core c = half c%2 of species c//2
    idx_c = []
    for e in range(E):
        ide = np.nonzero(spec == e)[0]
        half = (len(ide) + 1) // 2
        idx_c.append(ide[:half])
        idx_c.append(ide[half:])
    cap = max(128, -(-max(len(ids) for ids in idx_c) // 32) * 32)
    chunk_list = _chunks(cap)

    wts = [_pack_weights_one(W1[e], Wh[e]) for e in range(E)]
    biases = []
    for e in range(E):
        bias = np.zeros((128, 4, 1), np.float32)
        bias[:, 0:2, 0] = b1[e].reshape(H1CH, 128).T
        bhpad = np.zeros(H2CH * 128, np.float32)
        bhpad[:H2] = bh[e]
        bias[:, 2:4, 0] = bhpad.reshape(H2CH, 128).T
        biases.append(bias)

    rep_bf = rep.astype(BF16)
    in_maps = []
    for c in range(NCORES):
        e = c // 2
        ids = idx_c[c]
        xt = np.zeros((128, DCH * cap), BF16)
        for n0, n in chunk_list:
            sub = ids[n0:n0 + n]
            r = len(sub)
            if r == 0:
                continue
            blk = rep_bf[sub].reshape(r, DCH, 128).transpose(2, 1, 0)
            c0 = DCH * n0
            for d in range(DCH):
                xt[:, c0 + d * n: c0 + d * n + r] = blk[:, d, :]
        in_maps.append({"xt": xt, "wt": wts[e], "bias": biases[e]})

    if cap not in _graph_cache:
        nc = _build_graph(cap)
        nc.finalize()
        _graph_cache[cap] = nc
    nc = _graph_cache[cap]

    res = run_bass_kernel_spmd(nc, in_maps, core_ids=list(range(NCORES)))
    _last_run.update(nc=nc, in_maps=in_maps, cap=cap)

    # host combine: layer-3 dot + b2, scatter-add into molecules
    out = np.zeros(B, np.float64)
    for c in range(NCORES):
        e = c // 2
        ids = idx_c[c]
        nids = len(ids)
        if nids == 0:
            continue
        h2a = np.asarray(res.results[c]["out0"]).astype(np.float32)[:, :nids]
        h2b = np.asarray(res.results[c]["out1"]).astype(np.float32)[:, :nids]
        en = W2[e, :128] @ h2a + W2[e, 128:H2] @ h2b + b2[e]
        out += np.bincount(ids // A, weights=en.astype(np.float64), minlength=B)
    return out.astype(np.float32)

